# revision 7
# baseline (speedup 1.0000x reference)
"""Masked multi-head attention (sparse_attention) Trainium2 Bass kernel.

Data-parallel over batch: B=8 batch elements, one per NeuronCore.
Per-core computation for batch element b (all shapes hardcoded):
  x [1024,768], adj [1024,1024], Wq/Wk/Wv [768,768], bq/bk/bv [768], beta []
  q = x@Wq+bq; k = x@Wk+bk; v = x@Wv+bv      (12 heads of 64)
  S = q k^T / 8 + beta*adj ; masked where adj<=0 ; P = softmax(S)
  out = P v  -> [1024, 768]

Kernel strategy (per core):
  - X^T via PE transposes; Q^T,K^T = W^T-chunk matmuls (float32r, 1cyc/row)
    stored bf16 as [768,1024] so head-pair 2c,2c+1 sits in partition halves
    of tile c.  V stored bf16 as [1024, 12*65] with a ones column per head
    (the ones column makes the PV matmul emit softmax row-sums for free).
  - m^T = (adjT>0)*exp(beta*adjT) once per batch (shared by all 12 heads);
    then per head P^T = m^T * exp(S^T/8) with S^T = K_h @ Q_h^T (K=64 MMs).
    No max-subtraction needed: logits are O(1) for this problem.
  - out^T[65,512] = [V_h|1]^T @ P^T  (N=512 matmuls), PE-transpose back,
    reciprocal of column 64 (row-sum) scales the head output.
"""

import sys

import numpy as np

try:
    import concourse.bass as bass
except ImportError:  # container default location
    sys.path.insert(0, "/opt/trn_rl_repo")
    import concourse.bass as bass

from contextlib import ExitStack

import concourse.bacc as bacc
import concourse.mybir as mybir
import concourse.tile as tile
from concourse.bass_utils import run_bass_kernel_spmd
from concourse.masks import make_identity

B, N, D, H, HD = 8, 1024, 768, 12, 64
P = 128
NT = N // P  # 8 row chunks
DT = D // P  # 6 feature chunks
NH = 512  # free-dim tile for matmuls
HD1 = HD + 1  # head dim + ones column

F32 = mybir.dt.float32
F32R = mybir.dt.float32r
BF16 = mybir.dt.bfloat16
AF = mybir.ActivationFunctionType
ALU = mybir.AluOpType


def _emit(tc, ctx, x_d, adj_d, w_d, b_d, beta_d, out_d):
    nc = tc.nc

    const = ctx.enter_context(tc.tile_pool(name="const", bufs=1))
    ident = const.tile([P, P], F32, tag="ident")
    make_identity(nc, ident)
    def bcast(ap, n_part):
        return bass.AP(tensor=ap.tensor, offset=ap.offset, ap=[[0, n_part]] + list(ap.ap))

    beta_sb = const.tile([P, 1], F32, tag="beta")
    nc.gpsimd.dma_start(out=beta_sb, in_=bcast(beta_d[0], P))
    bq_sb = const.tile([P, DT], F32, tag="bq")
    nc.gpsimd.dma_start(out=bq_sb, in_=b_d["bq"].rearrange("(c p) -> p c", p=P))
    bk_sb = const.tile([P, DT], F32, tag="bk")
    nc.gpsimd.dma_start(out=bk_sb, in_=b_d["bk"].rearrange("(c p) -> p c", p=P))
    bv_bc = const.tile([P, D], F32, tag="bv")
    nc.gpsimd.dma_start(out=bv_bc, in_=bcast(b_d["bv"], P))

    # Persistent tensors (live across phases)
    pers = ctx.enter_context(tc.tile_pool(name="pers", bufs=1))
    qt = [pers.tile([P, N], BF16, tag=f"qt{c}", name=f"qt{c}") for c in range(DT)]
    kt = [pers.tile([P, N], BF16, tag=f"kt{c}", name=f"kt{c}") for c in range(DT)]
    v_sb = [pers.tile([P, H * HD1], BF16, tag=f"v{i}", name=f"v{i}") for i in range(NT)]
    m_sb = [pers.tile([P, N], BF16, tag=f"m{k}", name=f"m{k}") for k in range(NT)]
    out_sb = [pers.tile([P, D], F32, tag=f"os{i}", name=f"os{i}") for i in range(NT)]

    # ---------------- Phase 1+2: X^T and projections ----------------
    with tc.tile_pool(name="xw", bufs=1) as xw, \
         tc.tile_pool(name="pstr", space="PSUM", bufs=4) as pstr, \
         tc.tile_pool(name="psmm", space="PSUM", bufs=4) as psmm:
        x_sb = [xw.tile([P, D], F32, tag=f"x{i}", name=f"x{i}") for i in range(NT)]
        for i in range(NT):
            nc.sync.dma_start(out=x_sb[i], in_=x_d[i * P:(i + 1) * P, :])

        xt = [xw.tile([P, N], F32R, tag=f"xt{c}", name=f"xt{c}") for c in range(DT)]
        for c in range(DT):
            for i in range(NT):
                tp = pstr.tile([P, P], F32, tag="tp", name="tp")
                nc.tensor.transpose(tp, x_sb[i][:, c * P:(c + 1) * P], ident)
                nc.scalar.copy(xt[c][:, i * P:(i + 1) * P], tp)

        w_sb = {}
        for wname in ("wq", "wk", "wv"):
            w_sb[wname] = [
                xw.tile([P, D], F32R, tag=f"{wname}{c}", name=f"{wname}{c}")
                for c in range(DT)
            ]
            for c in range(DT):
                nc.sync.dma_start(out=w_sb[wname][c], in_=w_d[wname][c * P:(c + 1) * P, :])

        # Q^T, K^T: out[d_out, n] accumulated over d_in chunks; bias per-partition
        for wname, dst, bias_sb in (("wq", qt, bq_sb), ("wk", kt, bk_sb)):
            for c in range(DT):
                for qh in range(2):
                    ps = psmm.tile([P, NH], F32, tag="mm", name="mm")
                    for kc in range(DT):
                        nc.tensor.matmul(
                            ps,
                            lhsT=w_sb[wname][kc][:, c * P:(c + 1) * P],
                            rhs=xt[kc][:, qh * NH:(qh + 1) * NH],
                            start=(kc == 0),
                            stop=(kc == DT - 1),
                        )
                    nc.vector.tensor_scalar_add(
                        dst[c][:, qh * NH:(qh + 1) * NH], ps, bias_sb[:, c:c + 1]
                    )

        # V: out[n, d_out]; scatter into stride-65 per-head layout with bias
        for i in range(NT):
            for s, w in ((0, NH), (NH, D - NH)):
                ps = psmm.tile([P, w], F32, tag="mm", name="mm")
                for kc in range(DT):
                    nc.tensor.matmul(
                        ps,
                        lhsT=xt[kc][:, i * P:(i + 1) * P],
                        rhs=w_sb["wv"][kc][:, s:s + w],
                        start=(kc == 0),
                        stop=(kc == DT - 1),
                    )
                nh = w // HD
                h0 = s // HD
                dst3 = v_sb[i].rearrange("p (h j) -> p h j", j=HD1)[:, h0:h0 + nh, 0:HD]
                src3 = ps.rearrange("p (h j) -> p h j", j=HD)
                bias3 = bv_bc[:, s:s + w].rearrange("p (h j) -> p h j", j=HD)
                nc.vector.tensor_add(dst3, src3, bias3)
            ones3 = v_sb[i].rearrange("p (h j) -> p h j", j=HD1)[:, :, HD:HD1]
            nc.vector.memset(ones3, 1.0)

    # ---------------- Phase 3: mask m^T = (adjT>0)*exp(beta*adjT) ----------------
    with tc.tile_pool(name="adjp", bufs=1) as adjp, \
         tc.tile_pool(name="psadj", space="PSUM", bufs=4) as psadj, \
         tc.tile_pool(name="etp", bufs=4) as etp:
        adj_sb = [adjp.tile([P, N], F32, tag=f"adj{i}", name=f"adj{i}") for i in range(NT)]
        for i in range(NT):
            nc.sync.dma_start(out=adj_sb[i], in_=adj_d[i * P:(i + 1) * P, :])
        for k in range(NT):
            for j in range(0, NT, 4):
                aps = psadj.tile([P, 4 * P], F32, tag="aps", name="aps")
                for bb in range(4):
                    nc.tensor.transpose(
                        aps[:, bb * P:(bb + 1) * P],
                        adj_sb[j + bb][:, k * P:(k + 1) * P],
                        ident,
                    )
                e = etp.tile([P, 4 * P], BF16, tag="e", name="e")
                nc.scalar.activation(e, aps, AF.Exp, scale=beta_sb[:, 0:1])
                nc.vector.scalar_tensor_tensor(
                    out=m_sb[k][:, j * P:(j + 4) * P],
                    in0=aps,
                    scalar=0.0,
                    in1=e,
                    op0=ALU.is_gt,
                    op1=ALU.mult,
                )

    # ---------------- Phase 4: attention per head ----------------
    with tc.tile_pool(name="pp", bufs=2) as pp, \
         tc.tile_pool(name="etq", bufs=4) as etq, \
         tc.tile_pool(name="pss", space="PSUM", bufs=4) as pss, \
         tc.tile_pool(name="pso", space="PSUM", bufs=2) as pso, \
         tc.tile_pool(name="psf", space="PSUM", bufs=2) as psf, \
         tc.tile_pool(name="fin", bufs=4) as fin:
        for h in range(H):
            c, r0 = h // 2, (h % 2) * HD
            p_t = [pp.tile([P, N], BF16, tag=f"p{k}", name=f"p{k}") for k in range(NT)]
            for k in range(NT):
                for qh in range(2):
                    ps = pss.tile([P, NH], F32, tag="s", name="s")
                    nc.tensor.matmul(
                        ps,
                        lhsT=kt[c][r0:r0 + HD, k * P:(k + 1) * P],
                        rhs=qt[c][r0:r0 + HD, qh * NH:(qh + 1) * NH],
                        start=True,
                        stop=True,
                    )
                    e = etq.tile([P, NH], BF16, tag="et", name="et")
                    nc.scalar.activation(e, ps, AF.Exp, scale=0.125)
                    nc.vector.tensor_mul(
                        p_t[k][:, qh * NH:(qh + 1) * NH],
                        e,
                        m_sb[k][:, qh * NH:(qh + 1) * NH],
                    )
            for qh in range(2):
                ops = pso.tile([HD1, NH], F32, tag="ov", name="ov")
                for k in range(NT):
                    nc.tensor.matmul(
                        ops,
                        lhsT=v_sb[k][:, h * HD1:(h + 1) * HD1],
                        rhs=p_t[k][:, qh * NH:(qh + 1) * NH],
                        start=(k == 0),
                        stop=(k == NT - 1),
                    )
                ot = fin.tile([HD1, NH], F32, tag="ot", name="ot")
                if qh == 0:
                    nc.scalar.copy(ot, ops)
                else:
                    nc.vector.tensor_copy(ot, ops)
                for blk in range(4):
                    qc = qh * 4 + blk
                    fp = psf.tile([P, HD1], F32, tag="fp", name="fp")
                    nc.tensor.transpose(
                        fp, ot[:, blk * P:(blk + 1) * P], ident[0:HD1, 0:HD1]
                    )
                    rec = fin.tile([P, 1], F32, tag="rec", name="rec")
                    nc.vector.reciprocal(rec, fp[:, HD:HD1])
                    nc.vector.tensor_scalar_mul(
                        out_sb[qc][:, h * HD:(h + 1) * HD], fp[:, 0:HD], rec
                    )
        for i in range(NT):
            nc.sync.dma_start(out=out_d[i * P:(i + 1) * P, :], in_=out_sb[i])


def build_nc():
    nc = bacc.Bacc("TRN2", target_bir_lowering=False, debug=False, num_devices=B)
    x_d = nc.dram_tensor("x", [N, D], F32, kind="ExternalInput").ap()
    adj_d = nc.dram_tensor("adj", [N, N], F32, kind="ExternalInput").ap()
    w_d = {
        "wq": nc.dram_tensor("wq", [D, D], F32R, kind="ExternalInput").ap(),
        "wk": nc.dram_tensor("wk", [D, D], F32R, kind="ExternalInput").ap(),
        "wv": nc.dram_tensor("wv", [D, D], F32R, kind="ExternalInput").ap(),
    }
    b_d = {
        "bq": nc.dram_tensor("bq", [D], F32, kind="ExternalInput").ap(),
        "bk": nc.dram_tensor("bk", [D], F32, kind="ExternalInput").ap(),
        "bv": nc.dram_tensor("bv", [D], F32, kind="ExternalInput").ap(),
    }
    beta_d = nc.dram_tensor("beta", [1, 1], F32, kind="ExternalInput").ap()
    out_d = nc.dram_tensor("out", [N, D], F32, kind="ExternalOutput").ap()
    with tile.TileContext(nc) as tc, ExitStack() as ctx:
        _emit(tc, ctx, x_d, adj_d, w_d, b_d, beta_d, out_d)
    nc.compile()
    return nc


_CACHE = {}


def _get_nc():
    if "nc" not in _CACHE:
        _CACHE["nc"] = build_nc()
    return _CACHE["nc"]


def make_in_maps(input_graph, adj, Wq, bq, Wk, bk, Wv, bv, beta):
    f = lambda a: np.ascontiguousarray(np.asarray(a), dtype=np.float32)
    wq, wk, wv = f(Wq), f(Wk), f(Wv)
    bqa, bka, bva = f(bq), f(bk), f(bv)
    beta_a = f(beta).reshape(1, 1)
    ig, ad = f(input_graph), f(adj)
    return [
        {
            "x": ig[b], "adj": ad[b],
            "wq": wq, "wk": wk, "wv": wv,
            "bq": bqa, "bk": bka, "bv": bva,
            "beta": beta_a,
        }
        for b in range(B)
    ]


def run_hw(in_maps, **kwargs):
    nc = _get_nc()
    return run_bass_kernel_spmd(nc, in_maps, list(range(B)), **kwargs)


def kernel(input_graph, adj, Wq, bq, Wk, bk, Wv, bv, beta):
    in_maps = make_in_maps(input_graph, adj, Wq, bq, Wk, bk, Wv, bv, beta)
    res = run_hw(in_maps)
    return np.stack([res.results[i]["out"] for i in range(B)], axis=0).astype(np.float32)
